# revision 5
# baseline (speedup 1.0000x reference)
"""Min-Euclidean-distance retrieval kernel for Trainium2 (8 NeuronCores).

Reference computation:
    x: [1, 2048, 512], y: [1, 65536, 512] (fp32)
    sq[p, r] = ||x_p||^2 + ||y_r||^2 - 2 <x_p, y_r>
    out = min over (p, r) of sqrt(max(sq, 0))

Sharding: the candidate pool (R) is split across 8 cores, 8192 candidates
each. Host pre-transposes x and each y shard so both GEMM operands arrive
with the contraction dim (d) on SBUF partitions — no on-chip transposes.

Per core, per PSUM tile [128 candidates x 512 queries] (fp32r matmuls run
at full PE rate for moving free dim >= 256):
  ScalarE:  h = -2*G + y2[r]          (per-partition bias)
  VectorE:  acc = min(acc, h)         (elementwise across candidate tiles)
The per-query ||x_p||^2 term is constant across candidates, so it is added
on the host, along with the final min across lanes/cores and the
(monotone) sqrt.
"""

import sys

for _p in ("/opt/trn_rl_repo", "/root/.axon_site/_ro/trn_rl_repo"):
    if _p not in sys.path:
        sys.path.append(_p)

import numpy as np

import concourse.bass as bass
import concourse.mybir as mybir
import concourse.tile as tile
from concourse import bacc, bass_utils

P = 2048          # queries
R = 65536         # candidates (full)
D = 512           # feature dim
NCORES = 8
R_LOC = R // NCORES      # 8192 candidates per core
P_CHUNKS = P // 512      # 4 moving chunks of queries
R_TILES = R_LOC // 128   # 64 stationary tiles of candidates
K_TILES = D // 128       # 4 contraction tiles

F32 = mybir.dt.float32
F32R = mybir.dt.float32r


def _build_module() -> bass.Bass:
    nc = bacc.Bacc("TRN2", target_bir_lowering=False, debug=False)

    xt = nc.dram_tensor("xt", [D, P], F32R, kind="ExternalInput")
    yt = nc.dram_tensor("yt", [D, R_LOC], F32R, kind="ExternalInput")
    # y2t[lane, t] = ||y_r||^2 for r = t*128 + lane.
    y2t = nc.dram_tensor("y2t", [128, R_TILES], F32, kind="ExternalInput")
    # acc[lane, c*512 + j] = min over r-tiles t of
    #   (y2[t*128+lane] - 2 G[t*128+lane, c*512+j])
    out = nc.dram_tensor("out", [128, P], F32, kind="ExternalOutput")

    xt_ap = xt.ap().rearrange("(ko q) p -> q ko p", q=128)
    yt_ap = yt.ap().rearrange("(ko q) r -> q ko r", q=128)

    with tile.TileContext(nc) as tc:
        with (
            tc.tile_pool(name="big", bufs=1) as big,
            tc.tile_pool(name="scr", bufs=4) as scr,
            tc.tile_pool(name="psum", bufs=8, space="PSUM") as psum,
        ):
            xt_sb = big.tile([128, K_TILES, P], F32R)
            yt_sb = big.tile([128, K_TILES, R_LOC], F32R)
            y2t_sb = big.tile([128, R_TILES], F32)
            acc = big.tile([128, P], F32)

            nc.sync.dma_start(y2t_sb[:], y2t.ap())
            # Chunked loads so compute can start before everything lands.
            for c in range(P_CHUNKS):
                s = slice(c * 512, (c + 1) * 512)
                nc.sync.dma_start(xt_sb[:, :, s], xt_ap[:, :, s])
            for g in range(8):
                s = slice(g * 1024, (g + 1) * 1024)
                nc.sync.dma_start(yt_sb[:, :, s], yt_ap[:, :, s])

            for c in range(P_CHUNKS):
                ps = slice(c * 512, (c + 1) * 512)
                acc_c = acc[:, ps]
                for t in range(R_TILES):
                    pt = psum.tile([128, 512], F32, name="pt")
                    for k in range(K_TILES):
                        nc.tensor.matmul(
                            pt[:],
                            lhsT=yt_sb[:, k, t * 128 : (t + 1) * 128],
                            rhs=xt_sb[:, k, ps],
                            start=(k == 0),
                            stop=(k == K_TILES - 1),
                        )
                    bias = y2t_sb[:, t : t + 1]
                    if t == 0:
                        nc.scalar.activation(
                            out=acc_c,
                            in_=pt[:],
                            func=mybir.ActivationFunctionType.Identity,
                            bias=bias,
                            scale=-2.0,
                        )
                    else:
                        h = scr.tile([128, 512], F32, name="h")
                        nc.scalar.activation(
                            out=h[:],
                            in_=pt[:],
                            func=mybir.ActivationFunctionType.Identity,
                            bias=bias,
                            scale=-2.0,
                        )
                        nc.vector.tensor_tensor(
                            out=acc_c,
                            in0=acc_c,
                            in1=h[:],
                            op=mybir.AluOpType.min,
                        )
            nc.sync.dma_start(out.ap(), acc[:])
    nc.compile()
    return nc


_module_cache: bass.Bass | None = None


def _get_module() -> bass.Bass:
    global _module_cache
    if _module_cache is None:
        _module_cache = _build_module()
    return _module_cache


def _prepare_inputs(x: np.ndarray, y: np.ndarray):
    """Host-side sharding/layout prep. Returns per-core input maps."""
    xt = np.ascontiguousarray(x.T)  # [512, 2048]
    in_maps = []
    for c in range(NCORES):
        yc = y[c * R_LOC : (c + 1) * R_LOC]
        yct = np.ascontiguousarray(yc.T)  # [512, 8192]
        y2 = np.einsum("rd,rd->r", yc, yc, dtype=np.float32)
        y2t = np.ascontiguousarray(y2.reshape(R_TILES, 128).T)
        in_maps.append({"xt": xt, "yt": yct, "y2t": y2t})
    return in_maps


def _postprocess(x: np.ndarray, accs: np.ndarray) -> np.ndarray:
    """accs: [NCORES, 128, P] partial mins (missing the x2 term)."""
    m = accs.min(axis=(0, 1))  # [P]; min over cores and candidate lanes
    x2 = np.einsum("pd,pd->p", x, x, dtype=np.float32)
    sq_min = np.float32((x2 + m).min())
    return np.sqrt(np.maximum(sq_min, np.float32(0.0)), dtype=np.float32)


def kernel(
    predicted_transaction_company: np.ndarray,
    future_transaction_companies_inc_current_data: np.ndarray,
) -> np.ndarray:
    x = np.asarray(predicted_transaction_company, dtype=np.float32)[0]
    y = np.asarray(future_transaction_companies_inc_current_data, dtype=np.float32)[0]

    nc = _get_module()
    in_maps = _prepare_inputs(x, y)
    res = bass_utils.run_bass_kernel_spmd(nc, in_maps, core_ids=list(range(NCORES)))
    accs = np.stack([r["out"] for r in res.results])
    return _postprocess(x, accs)


# revision 9
# speedup vs baseline: 1.2494x; 1.2494x over previous
"""Min-Euclidean-distance retrieval kernel for Trainium2 (8 NeuronCores).

Reference computation:
    x: [1, 2048, 512], y: [1, 65536, 512] (fp32)
    sq[p, r] = ||x_p||^2 + ||y_r||^2 - 2 <x_p, y_r>
    out = min over (p, r) of sqrt(max(sq, 0))

Sharding: the candidate pool (R) is split across 8 cores, 8192 candidates
each. Host pre-transposes x and each y shard so both GEMM operands arrive
with the contraction dim (d) on SBUF partitions — no on-chip transposes.

Per core, per PSUM tile [128 candidates x 512 queries] (fp32r matmuls run
at full PE rate for moving free dim >= 256):
  ScalarE:  h = -2*G + y2[r]          (per-partition bias)
  VectorE:  acc = min(acc, h)         (elementwise across candidate tiles)
The per-query ||x_p||^2 term is constant across candidates, so it is added
on the host, along with the final min across lanes/cores and the
(monotone) sqrt.
"""

import sys

for _p in ("/opt/trn_rl_repo", "/root/.axon_site/_ro/trn_rl_repo"):
    if _p not in sys.path:
        sys.path.append(_p)

import ml_dtypes
import numpy as np

import concourse.bass as bass
import concourse.mybir as mybir
import concourse.tile as tile
from concourse import bacc, bass_utils

P = 2048          # queries
R = 65536         # candidates (full)
D = 512           # feature dim
NCORES = 8
R_LOC = R // NCORES      # 8192 candidates per core
P_CHUNKS = P // 512      # 4 moving chunks of queries
R_TILES = R_LOC // 128   # 64 stationary tiles of candidates
K_TILES = D // 128       # 4 contraction tiles

F32 = mybir.dt.float32
# GEMM operand dtype: bf16 runs the PE at 1 cycle/row with cheap weight
# loads; the resulting distance error is ~1e-4 relative, far inside
# tolerance (measured 8.4e-6 end-to-end with f32r, bf16 adds ~3e-5).
MM_DT = mybir.dt.bfloat16
MM_NP = ml_dtypes.bfloat16


def _build_module() -> bass.Bass:
    nc = bacc.Bacc("TRN2", target_bir_lowering=False, debug=False)

    xt = nc.dram_tensor("xt", [D, P], MM_DT, kind="ExternalInput")
    yt = nc.dram_tensor("yt", [D, R_LOC], MM_DT, kind="ExternalInput")
    # y2t[lane, t] = ||y_r||^2 for r = t*128 + lane.
    y2t = nc.dram_tensor("y2t", [128, R_TILES], F32, kind="ExternalInput")
    # acc[lane, c*512 + j] = min over r-tiles t of
    #   (y2[t*128+lane] - 2 G[t*128+lane, c*512+j])
    out = nc.dram_tensor("out", [128, P], F32, kind="ExternalOutput")

    xt_ap = xt.ap().rearrange("(ko q) p -> q ko p", q=128)
    yt_ap = yt.ap().rearrange("(ko q) r -> q ko r", q=128)

    with tile.TileContext(nc) as tc:
        with (
            tc.tile_pool(name="big", bufs=1) as big,
            tc.tile_pool(name="scr", bufs=4) as scr,
            tc.tile_pool(name="psum", bufs=8, space="PSUM") as psum,
        ):
            xt_sb = big.tile([128, K_TILES, P], MM_DT)
            yt_sb = big.tile([128, K_TILES, R_LOC], MM_DT)
            y2t_sb = big.tile([128, R_TILES], F32)
            acc = big.tile([128, P], F32)

            # Load order tuned so the first PSUM tile's operands (x chunk 0,
            # y tiles 0-7) land as early as possible.
            def load_x(c):
                s = slice(c * 512, (c + 1) * 512)
                nc.sync.dma_start(xt_sb[:, :, s], xt_ap[:, :, s])

            def load_y(g, ng=16):
                w = R_LOC // ng
                s = slice(g * w, (g + 1) * w)
                nc.sync.dma_start(yt_sb[:, :, s], yt_ap[:, :, s])

            load_x(0)
            load_y(0)
            nc.sync.dma_start(y2t_sb[:], y2t.ap())
            for g in range(1, 16):
                load_y(g)
            for c in range(1, P_CHUNKS):
                load_x(c)

            for c in range(P_CHUNKS):
                ps = slice(c * 512, (c + 1) * 512)
                acc_c = acc[:, ps]
                for t in range(R_TILES):
                    pt = psum.tile([128, 512], F32, name="pt")
                    for k in range(K_TILES):
                        nc.tensor.matmul(
                            pt[:],
                            lhsT=yt_sb[:, k, t * 128 : (t + 1) * 128],
                            rhs=xt_sb[:, k, ps],
                            start=(k == 0),
                            stop=(k == K_TILES - 1),
                        )
                    bias = y2t_sb[:, t : t + 1]
                    if t == 0:
                        nc.scalar.activation(
                            out=acc_c,
                            in_=pt[:],
                            func=mybir.ActivationFunctionType.Identity,
                            bias=bias,
                            scale=-2.0,
                        )
                    else:
                        h = scr.tile([128, 512], F32, name="h")
                        nc.scalar.activation(
                            out=h[:],
                            in_=pt[:],
                            func=mybir.ActivationFunctionType.Identity,
                            bias=bias,
                            scale=-2.0,
                        )
                        nc.vector.tensor_tensor(
                            out=acc_c,
                            in0=acc_c,
                            in1=h[:],
                            op=mybir.AluOpType.min,
                        )
            nc.sync.dma_start(out.ap(), acc[:])
    nc.compile()
    return nc


_module_cache: bass.Bass | None = None


def _get_module() -> bass.Bass:
    global _module_cache
    if _module_cache is None:
        _module_cache = _build_module()
    return _module_cache


def _prepare_inputs(x: np.ndarray, y: np.ndarray):
    """Host-side sharding/layout prep. Returns per-core input maps."""
    xt = np.ascontiguousarray(x.T.astype(MM_NP))  # [512, 2048]
    in_maps = []
    for c in range(NCORES):
        yc = y[c * R_LOC : (c + 1) * R_LOC]
        yct = np.ascontiguousarray(yc.T.astype(MM_NP))  # [512, 8192]
        y2 = np.einsum("rd,rd->r", yc, yc, dtype=np.float32)
        y2t = np.ascontiguousarray(y2.reshape(R_TILES, 128).T)
        in_maps.append({"xt": xt, "yt": yct, "y2t": y2t})
    return in_maps


def _postprocess(x: np.ndarray, accs: np.ndarray) -> np.ndarray:
    """accs: [NCORES, 128, P] partial mins (missing the x2 term)."""
    m = accs.min(axis=(0, 1))  # [P]; min over cores and candidate lanes
    x2 = np.einsum("pd,pd->p", x, x, dtype=np.float32)
    sq_min = np.float32((x2 + m).min())
    return np.sqrt(np.maximum(sq_min, np.float32(0.0)), dtype=np.float32)


def kernel(
    predicted_transaction_company: np.ndarray,
    future_transaction_companies_inc_current_data: np.ndarray,
) -> np.ndarray:
    x = np.asarray(predicted_transaction_company, dtype=np.float32)[0]
    y = np.asarray(future_transaction_companies_inc_current_data, dtype=np.float32)[0]

    nc = _get_module()
    in_maps = _prepare_inputs(x, y)
    res = bass_utils.run_bass_kernel_spmd(nc, in_maps, core_ids=list(range(NCORES)))
    accs = np.stack([r["out"] for r in res.results])
    return _postprocess(x, accs)
